# revision 15
# baseline (speedup 1.0000x reference)
"""Trainium2 Bass kernel for nn_MultiHeadAttention_81999515616076.

Reference computation (per batch b):
    xn = LN(x)                                    [N, IN]
    q  = xn @ W_q   -> [N, H, D]
    k,v= xn @ W_kv  -> [N, H, D] each
    ckv= LN(c_emb) @ W_ctx + b_ctx -> ck, cv      [M, D] (shared across heads)
    keys per head = [self keys (N)] + [null key] + [ctx keys (M)]  (2177 total)
    out = softmax(q.k / sqrt(D)) @ values         [N, H, D]
    y  = LN(out.reshape(N, H*D) @ W_out)          [N, IN]

Sharding (8 cores): core c -> batch b = c//4, head group g = c%4 (heads 4g..4g+3).
Per-core: LN+transpose of x, bf16 projections, flash-style attention for its 4
heads (scores computed transposed: [keys, tokens]; softmax denominator via a
ones-column in the PV matmul; no max subtraction -- scores are bounded ~N(0,0.4)),
out-projection partials, bf16 ReduceScatter(add) over the 4 cores of each batch,
and final LN on the received 512-token slice.  Host folds the input-LN gamma into
the projection weights, precomputes all LN-beta biases, and casts weights +
activations to bf16 (fp32 statistics / PSUM accumulation on device).
"""

import sys

sys.path.insert(0, "/opt/trn_rl_repo")

import numpy as np
import ml_dtypes

import concourse.bacc as bacc
import concourse.tile as tile
import concourse.mybir as mybir

B, N, IN = 2, 2048, 1024
H, D = 16, 64
CTX_DIM, M_CTX = 768, 128
NCORES = 8
HG = 4               # heads per core
FH = HG * D          # 256 local head-feats
BLK = 512            # token block
NBLK = N // BLK      # 4
KT = 17              # 16 self key tiles + 1 ctx key tile (null key handled separately)
SCALE = D ** -0.5    # 0.125
EPS = 1e-5

f32 = mybir.dt.float32
f32r = mybir.dt.float32r
bf16 = mybir.dt.bfloat16
i32 = mybir.dt.int32
AF = mybir.ActivationFunctionType
OP = mybir.AluOpType
BF = ml_dtypes.bfloat16


def build_program():
    nc = bacc.Bacc("TRN2", target_bir_lowering=False, debug=False, num_devices=NCORES)

    # ---- per-core DRAM tensors (values sharded/preprocessed by host) ----
    x_d = nc.dram_tensor("x_loc", [N, IN], bf16, kind="ExternalInput")
    wq_d = nc.dram_tensor("wq_loc", [IN, FH], bf16, kind="ExternalInput")
    wk_d = nc.dram_tensor("wk_loc", [IN, FH], bf16, kind="ExternalInput")
    wv_d = nc.dram_tensor("wv_loc", [IN, FH], bf16, kind="ExternalInput")
    wout_d = nc.dram_tensor("wout_loc", [FH, IN], bf16, kind="ExternalInput")
    wctx_d = nc.dram_tensor("wctx", [CTX_DIM, 2 * D], bf16, kind="ExternalInput")
    cemb_d = nc.dram_tensor("cemb_loc", [M_CTX, CTX_DIM], bf16, kind="ExternalInput")
    nullkv_d = nc.dram_tensor("nullkv", [2, D], bf16, kind="ExternalInput")
    qkb_d = nc.dram_tensor("qk_bias", [4, 128], f32, kind="ExternalInput")
    cvb_d = nc.dram_tensor("cv_bias", [1, FH], bf16, kind="ExternalInput")
    ckvb_d = nc.dram_tensor("ckv_bias", [128], f32, kind="ExternalInput")
    outg_d = nc.dram_tensor("out_g", [IN], bf16, kind="ExternalInput")
    outb_d = nc.dram_tensor("out_b", [IN], bf16, kind="ExternalInput")
    ident_d = nc.dram_tensor("const_ident", [128, 128], bf16, kind="ExternalInput")
    ones_d = nc.dram_tensor("const_ones", [1, 128], bf16, kind="ExternalInput")
    onesf_d = nc.dram_tensor("const_ones_f32", [1, 64], f32, kind="ExternalInput")
    y_out_d = nc.dram_tensor("y_out", [BLK, IN], bf16, kind="ExternalOutput")
    # internal DRAM for the collective (per-block to avoid WAR hazards)
    ypart_d = [nc.dram_tensor(f"y_partial{b}", [BLK, IN], bf16) for b in range(NBLK)]
    yred_d = [nc.dram_tensor(f"y_red{b}", [128, IN], bf16) for b in range(3)] + [
        [nc.dram_tensor(f"y_red3_{a}", [64, IN], bf16) for a in range(2)]]

    with tile.TileContext(nc) as tc:
        _emit(nc, tc, locals())
    nc.compile()
    return nc


def _emit(nc, tc, t):
    from contextlib import ExitStack

    x_d, cemb_d = t["x_d"], t["cemb_d"]
    wq_d, wk_d, wv_d, wout_d, wctx_d = t["wq_d"], t["wk_d"], t["wv_d"], t["wout_d"], t["wctx_d"]
    nullkv_d = t["nullkv_d"]
    qkb_d, cvb_d, ckvb_d = t["qkb_d"], t["cvb_d"], t["ckvb_d"]
    outg_d, outb_d = t["outg_d"], t["outb_d"]
    y_out_d, ypart_d, yred_d = t["y_out_d"], t["ypart_d"], t["yred_d"]
    ident_d, ones_d, onesf_d = t["ident_d"], t["ones_d"], t["onesf_d"]

    with ExitStack() as ctx:
        persist = ctx.enter_context(tc.tile_pool(name="persist", bufs=1))
        stat = ctx.enter_context(tc.tile_pool(name="stat", bufs=4))

        # ---------------- Phase 0: constants & weights ----------------
        ident = persist.tile([128, 128], bf16, name="ident", tag="ident")
        nc.sync.dma_start(ident, ident_d.ap())
        eps_t = persist.tile([128, 1], f32, name="eps", tag="eps")
        nc.vector.memset(eps_t, EPS)
        magic_t = persist.tile([128, 1], i32, name="magic", tag="magic")
        nc.vector.memset(magic_t, 0x5F3759DF)

        # host-precomputed per-partition biases
        qkb_sb = persist.tile([128, 4], f32, name="qkb_sb", tag="qkb_sb")
        nc.sync.dma_start(qkb_sb, qkb_d.ap().rearrange("a p -> p a"))
        ckvb_sb = persist.tile([128, 1], f32, name="ckvb_sb", tag="ckvb_sb")
        nc.sync.dma_start(ckvb_sb, ckvb_d.ap().rearrange("(a p) -> p a", p=128))

        ones_ap = ones_d.ap()
        ones_r = persist.tile([1, 128], bf16, name="ones_r", tag="ones_r")
        nc.sync.dma_start(ones_r, ones_ap)
        ones2 = persist.tile([65, 64], f32r, name="ones2", tag="ones2")
        nc.sync.dma_start(ones2[64:65, :], onesf_d.ap().bitcast(f32r))
        ones_hg = persist.tile([128, HG], bf16, name="ones_hg", tag="ones_hg")
        nc.sync.dma_start(ones_hg, ones_ap[0:1, 0:HG].to_broadcast([128, HG]))

        # null key/value: knull2 rows 0:64 and 64:128 both = null_k (for the two
        # row-packed head positions); nullv2 rows 0 = [null_v | 1].
        knull2 = persist.tile([128, 1], bf16, name="knull2", tag="knull2")
        nk_ap = nullkv_d.ap()[0:1, :].rearrange("a b -> b a")
        nc.sync.dma_start(knull2[0:64, :], nk_ap)
        nc.sync.dma_start(knull2[64:128, :], nk_ap)
        nullv2 = persist.tile([1, 65], bf16, name="nullv2", tag="nullv2")
        nc.sync.dma_start(nullv2[0:1, 0:64], nullkv_d.ap()[1:2, :])
        nc.sync.dma_start(nullv2[0:1, 64:65], ones_ap[0:1, 0:1])

        # Heavy P0 (weights + context projection), emitted AFTER block-0's LN/transpose
        # chains so the first x tiles hit the DMA queue first.
        wq_sb, wk_sb, wv_sb, wctx_sb, wout_sb = [], [], [], [], []
        cv_row = persist.tile([1, FH], bf16, name="cv_row", tag="cv_row")
        nc.sync.dma_start(cv_row, cvb_d.ap())
        ckvT_sb = persist.tile([128, M_CTX], bf16, name="ckvT", tag="ckvT")
        ck2 = persist.tile([128, M_CTX], bf16, name="ck2", tag="ck2")
        cv_ext = persist.tile([128, 65], bf16, name="cv_ext", tag="cv_ext")

        def emit_p0_heavy(p0sb, psT):
            for name, dram, lst in (("wq", wq_d, wq_sb), ("wk", wk_d, wk_sb), ("wv", wv_d, wv_sb)):
                for c in range(8):
                    w = persist.tile([128, FH], bf16, name=f"{name}{c}", tag=f"{name}{c}")
                    nc.sync.dma_start(w, dram.ap()[128 * c : 128 * (c + 1), :])
                    lst.append(w)
            for c in range(6):
                w = persist.tile([128, 2 * D], bf16, name=f"wctx{c}", tag=f"wctx{c}")
                nc.sync.dma_start(w, wctx_d.ap()[128 * c : 128 * (c + 1), :])
                wctx_sb.append(w)
            for c in range(2):
                w = persist.tile([128, IN], bf16, name=f"wout{c}", tag=f"wout{c}")
                nc.sync.dma_start(w, wout_d.ap()[128 * c : 128 * (c + 1), :])
                wout_sb.append(w)
            # ---- context projection: ckv^T = W_ctx'.T @ LN(c_emb).T + bias ----
            cemb_sb = p0sb.tile([128, CTX_DIM], bf16, name="cemb", tag="cemb")
            nc.sync.dma_start(cemb_sb, cemb_d.ap())
            stc = stat.tile([128, 3, 6], f32, name="stc", tag="stc")
            for i in range(3):
                nc.vector.bn_stats(stc[:, i, :], cemb_sb[:, 256 * i : 256 * (i + 1)])
            mvc = stat.tile([128, 2], f32, name="mvc", tag="mvc")
            nc.vector.bn_aggr(mvc, stc)
            stdc = stat.tile([128, 1], f32, name="stdc", tag="stdc")
            nc.scalar.activation(stdc, mvc[:, 1:2], AF.Sqrt, bias=eps_t[:, 0:1])
            rstd_c = stat.tile([128, 1], f32, name="rstd_c", tag="rstd_c")
            nc.vector.reciprocal_approx_fast(rstd_c, stdc)
            zc = p0sb.tile([128, CTX_DIM], bf16, name="zc", tag="zc")
            nc.vector.tensor_scalar(zc, cemb_sb, mvc[:, 0:1], rstd_c, op0=OP.subtract, op1=OP.mult)
            tpc = psT.tile([128, CTX_DIM], bf16, name="tpc", tag="tp")
            for c in range(6):
                nc.tensor.transpose(tpc[:, 128 * c : 128 * (c + 1)], zc[:, 128 * c : 128 * (c + 1)], ident)
            zcT = p0sb.tile([128, 6, 128], bf16, name="zcT", tag="zcT")
            nc.any.tensor_copy(zcT, tpc.rearrange("p (c w) -> p c w", c=6))
            psk = psT.tile([128, M_CTX], f32, name="psk", tag="tp")
            for c in range(6):
                nc.tensor.matmul(psk, wctx_sb[c], zcT[:, c, :], start=(c == 0), stop=(c == 5))
            nc.vector.tensor_scalar_add(ckvT_sb, psk, ckvb_sb[:, 0:1])
            # ck duplicated into both row-halves (for 2-head row packing)
            nc.sync.dma_start(ck2[0:64, :], ckvT_sb[0:64, :])
            nc.sync.dma_start(ck2[64:128, :], ckvT_sb[0:64, :])
            # cv in normal layout [M_CTX, 64] with a ones column -> [128, 65]
            cvT_tmp = p0sb.tile([64, M_CTX], bf16, name="cvT_tmp", tag="cvT_tmp")
            nc.sync.dma_start(cvT_tmp, ckvT_sb[64:128, :])
            ps_cv = psT.tile([128, 64], bf16, name="ps_cv", tag="tp")
            nc.tensor.transpose(ps_cv, cvT_tmp, ident[0:64, 0:64])
            nc.any.tensor_copy(cv_ext[:, 0:64], ps_cv)
            nc.vector.tensor_copy(cv_ext[:, 64:65], ones_hg[:, 0:1])

        # ---------------- persistent activation tensors ----------------
        qT = [persist.tile([128, N], bf16, name=f"qT{j}", tag=f"qT{j}") for j in range(2)]
        kT = [persist.tile([128, N], bf16, name=f"kT{j}", tag=f"kT{j}") for j in range(2)]
        attnT = [persist.tile([128, N], bf16, name=f"attnT{j}", tag=f"attnT{j}") for j in range(2)]
        v_tiles = []
        for i in range(16):
            vt = persist.tile([128, HG, 65], bf16, name=f"v{i}", tag=f"v{i}")
            nc.vector.tensor_copy(vt[:, :, 64:65], ones_hg.unsqueeze(2))
            v_tiles.append(vt)

        # ---------------- Phase 1: LN(x), transpose, q/k/v projections ----------------
        with tc.tile_pool(name="xp", bufs=3) as xp, \
             tc.tile_pool(name="zp", bufs=2) as zp, \
             tc.tile_pool(name="ztp", bufs=2) as ztp, \
             tc.tile_pool(name="p0sb", bufs=2) as p0sb, \
             tc.tile_pool(name="tpp", bufs=2, space="PSUM") as tpp, \
             tc.tile_pool(name="projp", bufs=2, space="PSUM") as projp, \
             tc.tile_pool(name="vpp", bufs=2, space="PSUM") as vpp:

            def emit_tts(blk):
                zT = ztp.tile([128, 8, BLK], bf16, name="zT", tag="zT")
                for tt in range(4):
                    t0 = BLK * blk + 128 * tt
                    x_t = xp.tile([128, IN], bf16, name="x_t", tag="x_t")
                    nc.sync.dma_start(x_t, x_d.ap()[t0 : t0 + 128, :])
                    st = stat.tile([128, 2, 6], f32, name="st", tag="st")
                    nc.vector.bn_stats(st[:, 0, :], x_t[:, 0:512])
                    nc.vector.bn_stats(st[:, 1, :], x_t[:, 512:1024])
                    mv = stat.tile([128, 2], f32, name="mv", tag="mv")
                    nc.vector.bn_aggr(mv, st)
                    sd = stat.tile([128, 1], f32, name="sd", tag="sd")
                    nc.scalar.activation(sd, mv[:, 1:2], AF.Sqrt, bias=eps_t[:, 0:1])
                    rstd = stat.tile([128, 1], f32, name="rstd", tag="rstd")
                    nc.vector.reciprocal_approx_fast(rstd, sd)
                    z_t = zp.tile([128, IN], bf16, name="z_t", tag="z_t")
                    nc.any.tensor_scalar(z_t, x_t, mv[:, 0:1], rstd, op0=OP.subtract, op1=OP.mult)
                    tp = tpp.tile([128, 1024], bf16, name="tp", tag="tp")
                    for c in range(8):
                        nc.tensor.transpose(tp[:, 128 * c : 128 * (c + 1)], z_t[:, 128 * c : 128 * (c + 1)], ident)
                    tpr = tp.rearrange("p (c w) -> p c w", c=8)
                    if tt % 2:
                        nc.scalar.activation(zT[:, :, 128 * tt : 128 * (tt + 1)], tpr, AF.Copy)
                    else:
                        nc.vector.tensor_copy(zT[:, :, 128 * tt : 128 * (tt + 1)], tpr)
                return zT

            def emit_proj(blk, zT):
                # q/k projections (transposed layout), per head-pair j
                for wi, (wsb, dst) in enumerate(((wq_sb, qT), (wk_sb, kT))):
                    for j in range(2):
                        ps = projp.tile([128, BLK], f32, name="proj", tag="proj")
                        for c in range(8):
                            nc.tensor.matmul(ps, wsb[c][:, 128 * j : 128 * (j + 1)], zT[:, c, :],
                                             start=(c == 0), stop=(c == 7))
                        nc.scalar.activation(dst[j][:, BLK * blk : BLK * (blk + 1)], ps, AF.Identity,
                                             bias=qkb_sb[:, 2 * wi + j : 2 * wi + j + 1])
                # v projection (normal layout) per 128-token tile
                for tt in range(4):
                    psv = vpp.tile([128, FH], f32, name="psv", tag="psv")
                    for c in range(8):
                        nc.tensor.matmul(psv, zT[:, c, 128 * tt : 128 * (tt + 1)], wv_sb[c],
                                         start=(c == 0), stop=False)
                    nc.tensor.matmul(psv, ones_r, cv_row, start=False, stop=True)
                    vt = v_tiles[4 * blk + tt]
                    nc.scalar.activation(vt[:, :, 0:64], psv.rearrange("p (h d) -> p h d", h=HG), AF.Copy)

            zT0 = emit_tts(0)
            emit_p0_heavy(p0sb, tpp)
            emit_proj(0, zT0)
            for blk in range(1, NBLK):
                zTb = emit_tts(blk)
                emit_proj(blk, zTb)

        # ---------------- Phases 2-4: attention, out-proj, chunked RS + final LN ----------------
        gout_rep = persist.tile([128, IN], bf16, name="gout_rep", tag="gout_rep")
        nc.sync.dma_start(gout_rep, outg_d.ap().unsqueeze(0).to_broadcast([128, IN]))
        bout_rep = persist.tile([128, IN], bf16, name="bout_rep", tag="bout_rep")
        nc.sync.dma_start(bout_rep, outb_d.ap().unsqueeze(0).to_broadcast([128, IN]))
        with tc.tile_pool(name="wtp", bufs=2) as wtp, \
             tc.tile_pool(name="oddp", bufs=2) as oddp, \
             tc.tile_pool(name="rcpp", bufs=2) as rcpp, \
             tc.tile_pool(name="expnp", bufs=2) as expnp, \
             tc.tile_pool(name="ysb", bufs=3) as ysbp, \
             tc.tile_pool(name="agp", bufs=2) as agp, \
             tc.tile_pool(name="fin", bufs=2) as fin, \
             tc.tile_pool(name="s0p", bufs=3, space="PSUM") as s0p, \
             tc.tile_pool(name="pvp", bufs=2, space="PSUM") as pvp:
            deferred = []
            deferred_fin = []

            def make_final_ln(blk, a, rows):
                # final LN on `rows` received token rows.  rstd via a
                # Quake-seeded Newton rsqrt on DVE (no scalar-engine table
                # swap mid-Exp); normalize chain in bf16 for DVE 2x mode.
                src_d = yred_d[blk] if a is None else yred_d[blk][a]
                row0 = 128 * blk + (0 if a is None else 64 * a)
                def final_ln():
                    yr = fin.tile([rows, IN], bf16, name="yr", tag="yr", bufs=4)
                    nc.gpsimd.dma_start(yr, src_d.ap())
                    st = stat.tile([rows, 2, 6], f32, name="st", tag="st")
                    nc.vector.bn_stats(st[:, 0, :], yr[:, 0:512])
                    nc.vector.bn_stats(st[:, 1, :], yr[:, 512:1024])
                    mv = stat.tile([rows, 2], f32, name="mv", tag="mv")
                    nc.vector.bn_aggr(mv, st)
                    ve = stat.tile([rows, 1], f32, name="ve", tag="ve")
                    nc.vector.tensor_scalar_add(ve, mv[:, 1:2], EPS)
                    t1 = stat.tile([rows, 1], i32, name="t1", tag="t1")
                    nc.vector.tensor_scalar(t1, ve.bitcast(i32), 1, None, op0=OP.arith_shift_right)
                    rstd = fin.tile([rows, 1], f32, name="rstd", tag="rstd", bufs=4)
                    nc.vector.tensor_tensor(rstd.bitcast(i32), magic_t[0:rows, :], t1, op=OP.subtract)
                    nr = stat.tile([rows, 1], f32, name="nr", tag="nr")
                    for _ in range(2):
                        nc.vector.tensor_tensor(nr, rstd, rstd, op=OP.mult)
                        nc.vector.tensor_tensor(nr, nr, ve, op=OP.mult)
                        nc.vector.tensor_scalar(nr, nr, -0.5, 1.5, op0=OP.mult, op1=OP.add)
                        nc.vector.tensor_tensor(rstd, rstd, nr, op=OP.mult)
                    zf = fin.tile([rows, IN], bf16, name="zf", tag="zf", bufs=4)
                    nc.vector.tensor_scalar(zf, yr, mv[:, 0:1], rstd, op0=OP.subtract, op1=OP.mult)
                    nc.vector.tensor_tensor(zf, zf, gout_rep[0:rows, :], op=OP.mult)
                    nc.vector.tensor_tensor(zf, zf, bout_rep[0:rows, :], op=OP.add)
                    nc.gpsimd.dma_start(y_out_d.ap()[row0 : row0 + rows, :], zf)
                return final_ln

            for blk in range(NBLK):
                bsl = slice(BLK * blk, BLK * (blk + 1))
                for pj in range(2):
                    q0 = qT[pj][0:64, bsl]
                    q1 = qT[pj][64:128, bsl]
                    # null-key scores for both heads -> one psum row, one exp
                    expn = expnp.tile([1, 2 * BLK], bf16, name="expn", tag="expn")
                    ps_nl = s0p.tile([1, 2 * BLK], f32, name="ps_nl", tag="ps_s")
                    nc.tensor.matmul(ps_nl[0:1, 0:BLK], knull2[0:64, :], q0, start=True, stop=True)
                    nc.tensor.matmul(ps_nl[0:1, BLK : 2 * BLK], knull2[64:128, :], q1, start=True,
                                     stop=True, tile_position=(64, 0))
                    nc.scalar.activation(expn, ps_nl, AF.Exp, scale=SCALE)
                    # scores -> exp -> PV, pipelined per key tile; both heads share one
                    # [128,1024] scores psum + one exp op (h0 cols 0:512, h1 cols 512:1024).
                    # PV trails one key tile behind so PE never head-of-line blocks on exp.
                    ps_pv0 = pvp.tile([65, BLK], f32, name="ps_pv0", tag="ps_pv")
                    ps_pv1 = pvp.tile([65, BLK], f32, name="ps_pv1", tag="ps_pv")

                    def pv_step(kt, wt):
                        lv0 = cv_ext[:, 0:65] if kt == 16 else v_tiles[kt][:, 2 * pj, :]
                        lv1 = cv_ext[:, 0:65] if kt == 16 else v_tiles[kt][:, 2 * pj + 1, :]
                        nc.tensor.matmul(ps_pv0, lv0, wt[:, 0:BLK], start=(kt == 0), stop=False)
                        nc.tensor.matmul(ps_pv1, lv1, wt[:, BLK : 2 * BLK], start=(kt == 0), stop=False)

                    pending = []
                    for kt in range(KT):
                        if kt == 2 and deferred:
                            deferred.pop(0)()
                        if blk == 3 and pj == 1 and kt == 12 and deferred_fin:
                            deferred_fin.pop(0)()
                        ps_s = s0p.tile([128, 2 * BLK], f32, name="ps_s", tag="ps_s")
                        wt = wtp.tile([128, 2 * BLK], bf16, name="wt", tag="wt", bufs=5)
                        l0 = ck2[0:64, :] if kt == 16 else kT[pj][0:64, 128 * kt : 128 * (kt + 1)]
                        l1 = ck2[64:128, :] if kt == 16 else kT[pj][64:128, 128 * kt : 128 * (kt + 1)]
                        nc.tensor.matmul(ps_s[:, 0:BLK], l0, q0, start=True, stop=True)
                        nc.tensor.matmul(ps_s[:, BLK : 2 * BLK], l1, q1, start=True, stop=True,
                                         tile_position=(64, 0))
                        if len(pending) >= 3:
                            pv_step(*pending.pop(0))
                        nc.scalar.activation(wt, ps_s, AF.Exp, scale=SCALE)
                        pending.append((kt, wt))
                    for args in pending:
                        pv_step(*args)
                    nc.tensor.matmul(ps_pv0, nullv2[0:1, :], expn[0:1, 0:BLK], start=False, stop=True)
                    nc.tensor.matmul(ps_pv1, nullv2[0:1, :], expn[0:1, BLK : 2 * BLK], start=False, stop=True)

                    # normalize: attnT = pv[0:64] * broadcast(1/denominator).  The recip
                    # (DVE) is emitted now so it overlaps the next pair's scores; the PE
                    # broadcast + multiply are deferred into the next pair's kt loop so
                    # the PE stream never head-of-line blocks on the DVE chain.
                    rcps = []
                    for h, ps_pv in ((0, ps_pv0), (1, ps_pv1)):
                        rcp = rcpp.tile([65, BLK], f32r, name="rcp", tag="rcp")
                        with nc.allow_low_precision(reason="fp32r recip of softmax denom"):
                            nc.vector.reciprocal(rcp[64:65, :], ps_pv[64:65, :])
                        rcps.append(rcp)

                    def do_norm(pj=pj, bsl=bsl, pvs=(ps_pv0, ps_pv1), rcps=tuple(rcps)):
                        for h, (ps_pv, rcp) in enumerate(zip(pvs, rcps)):
                            ps_rb = s0p.tile([64, BLK], f32, name="ps_rb", tag="ps_s")
                            nc.tensor.matmul(ps_rb, ones2[64:65, :], rcp[64:65, :],
                                             start=True, stop=True, tile_position=(64, 0))
                            rb_sb = rcpp.tile([64, BLK], f32, name="rb_sb", tag="rb_sb")
                            nc.vector.tensor_copy(rb_sb, ps_rb)
                            if h == 0:
                                nc.vector.tensor_tensor(attnT[pj][0:64, bsl], ps_pv[0:64, :], rb_sb, op=OP.mult)
                            else:
                                tmp = oddp.tile([64, BLK], bf16, name="odd", tag="odd")
                                nc.vector.tensor_tensor(tmp, ps_pv[0:64, :], rb_sb, op=OP.mult)
                                nc.sync.dma_start(attnT[pj][64:128, bsl], tmp)

                    deferred.append(do_norm)
                # flush pending normalizations, then out-projection for this block
                while deferred:
                    deferred.pop(0)()
                for tt4 in range(4):
                    tt = 4 * blk + tt4
                    y_sb = ysbp.tile([128, IN], bf16, name="y_sb", tag="y_sb")
                    for nh in range(2):
                        ps_y = s0p.tile([128, 512], f32, name="ps_y", tag="ps_s")
                        for c in range(2):
                            nc.tensor.matmul(ps_y, attnT[c][:, 128 * tt : 128 * (tt + 1)],
                                             wout_sb[c][:, 512 * nh : 512 * (nh + 1)],
                                             start=(c == 0), stop=(c == 1))
                        nc.vector.tensor_copy(y_sb[:, 512 * nh : 512 * (nh + 1)], ps_y)
                    nc.sync.dma_start(ypart_d[blk].ap()[128 * tt4 : 128 * (tt4 + 1), :], y_sb)
                    # blocks 0-2: one ReduceScatter per block; block 3: two
                    # half RS so the tail only waits on the last 256 rows
                    if blk < 3 and tt4 == 3:
                        nc.gpsimd.collective_compute(
                            "ReduceScatter",
                            OP.add,
                            replica_groups=[[0, 1, 2, 3], [4, 5, 6, 7]],
                            ins=[ypart_d[blk].ap()],
                            outs=[yred_d[blk].ap()],
                        )
                        deferred_fin.append(make_final_ln(blk, None, 128))
                    elif blk == 3 and tt4 in (1, 3):
                        a = tt4 // 2
                        nc.gpsimd.collective_compute(
                            "ReduceScatter",
                            OP.add,
                            replica_groups=[[0, 1, 2, 3], [4, 5, 6, 7]],
                            ins=[ypart_d[blk].ap()[256 * a : 256 * (a + 1), :]],
                            outs=[yred_d[blk][a].ap()],
                        )
                        deferred_fin.append(make_final_ln(blk, a, 64))
            while deferred_fin:
                deferred_fin.pop(0)()


def shard_inputs(inputs):
    """Split full inputs into 8 per-core input maps (host-side LN-gamma folding,
    bias precompute, bf16 casts)."""
    f = lambda v: np.asarray(v, np.float32)
    x = f(inputs["x"])
    c_emb = f(inputs["c_emb"])
    ln_g, ln_b = f(inputs["ln_g"]), f(inputs["ln_b"])
    ctx_g, ctx_b = f(inputs["ctx_ln_g"]), f(inputs["ctx_ln_b"])
    W_q = (ln_g[:, None] * f(inputs["W_q"])).reshape(IN, H, D)
    W_kv = (ln_g[:, None] * f(inputs["W_kv"])).reshape(IN, 2, H, D)
    W_ctx = ctx_g[:, None] * f(inputs["W_ctx"])
    W_out = f(inputs["W_out"]).reshape(H, D, IN)
    q_bias = (ln_b @ W_q.reshape(IN, H * D)).reshape(H, D)
    kv_bias = (ln_b @ W_kv.reshape(IN, 2 * H * D)).reshape(2, H, D)
    ckv_bias = ctx_b @ W_ctx + f(inputs["b_ctx"])
    common = {
        "const_ident": np.eye(128, dtype=BF),
        "const_ones": np.ones((1, 128), BF),
        "const_ones_f32": np.ones((1, 64), np.float32),
        "wctx": np.ascontiguousarray(W_ctx.astype(BF)),
        "nullkv": f(inputs["null_kv"]).astype(BF),
        "ckv_bias": np.ascontiguousarray(ckv_bias, dtype=np.float32),
        "out_g": f(inputs["out_ln_g"]).astype(BF),
        "out_b": f(inputs["out_ln_b"]).astype(BF),
    }
    in_maps = []
    for c in range(NCORES):
        b, g = c // 4, c % 4
        hs = slice(HG * g, HG * (g + 1))
        qkb = np.stack([q_bias[hs].reshape(FH)[0:128], q_bias[hs].reshape(FH)[128:256],
                        kv_bias[0, hs].reshape(FH)[0:128], kv_bias[0, hs].reshape(FH)[128:256]])
        in_maps.append({
            "x_loc": x[b].astype(BF),
            "cemb_loc": c_emb[b].astype(BF),
            "wq_loc": np.ascontiguousarray(W_q[:, hs].reshape(IN, FH).astype(BF)),
            "wk_loc": np.ascontiguousarray(W_kv[:, 0, hs].reshape(IN, FH).astype(BF)),
            "wv_loc": np.ascontiguousarray(W_kv[:, 1, hs].reshape(IN, FH).astype(BF)),
            "wout_loc": np.ascontiguousarray(W_out[hs].reshape(FH, IN).astype(BF)),
            "qk_bias": np.ascontiguousarray(qkb, dtype=np.float32),
            "cv_bias": np.ascontiguousarray(kv_bias[1, hs].reshape(1, FH).astype(BF)),
            **common,
        })
    return in_maps


def unshard(results):
    out = np.empty((B, N, IN), np.float32)
    for c in range(NCORES):
        b, r = c // 4, c % 4
        y = np.asarray(results[c]["y_out"], dtype=np.float32)
        for blk in range(3):
            t0 = BLK * blk + 128 * r
            out[b, t0 : t0 + 128, :] = y[128 * blk : 128 * (blk + 1)]
        for a in range(2):
            t0 = BLK * 3 + 256 * a + 64 * r
            y0 = 384 + 64 * a
            out[b, t0 : t0 + 64, :] = y[y0 : y0 + 64]
    return out


_CACHE = {}


def kernel(**inputs) -> np.ndarray:
    from concourse.bass_utils import run_bass_kernel_spmd

    if "nc" not in _CACHE:
        _CACHE["nc"] = build_program()
    nc = _CACHE["nc"]
    in_maps = shard_inputs(inputs)
    res = run_bass_kernel_spmd(nc, in_maps, list(range(NCORES))).results
    return unshard(res)


if __name__ == "__main__":
    nc = build_program()
    print("program built OK;",
          sum(1 for _ in nc.inst_map), "instructions")


# revision 17
# speedup vs baseline: 1.0281x; 1.0281x over previous
"""Trainium2 Bass kernel for nn_MultiHeadAttention_81999515616076.

Reference computation (per batch b):
    xn = LN(x)                                    [N, IN]
    q  = xn @ W_q   -> [N, H, D]
    k,v= xn @ W_kv  -> [N, H, D] each
    ckv= LN(c_emb) @ W_ctx + b_ctx -> ck, cv      [M, D] (shared across heads)
    keys per head = [self keys (N)] + [null key] + [ctx keys (M)]  (2177 total)
    out = softmax(q.k / sqrt(D)) @ values         [N, H, D]
    y  = LN(out.reshape(N, H*D) @ W_out)          [N, IN]

Sharding (8 cores): core c -> batch b = c//4, head group g = c%4 (heads 4g..4g+3).
Per-core: LN+transpose of x, bf16 projections, flash-style attention for its 4
heads (scores computed transposed: [keys, tokens]; softmax denominator via a
ones-column in the PV matmul; no max subtraction -- scores are bounded ~N(0,0.4)),
out-projection partials, bf16 ReduceScatter(add) over the 4 cores of each batch,
and final LN on the received 512-token slice.  Host folds the input-LN gamma into
the projection weights, precomputes all LN-beta biases, and casts weights +
activations to bf16 (fp32 statistics / PSUM accumulation on device).
"""

import sys

sys.path.insert(0, "/opt/trn_rl_repo")

import numpy as np
import ml_dtypes

import concourse.bacc as bacc
import concourse.tile as tile
import concourse.mybir as mybir

B, N, IN = 2, 2048, 1024
H, D = 16, 64
CTX_DIM, M_CTX = 768, 128
NCORES = 8
HG = 4               # heads per core
FH = HG * D          # 256 local head-feats
BLK = 512            # token block
NBLK = N // BLK      # 4
KT = 17              # 16 self key tiles + 1 ctx key tile (null key handled separately)
SCALE = D ** -0.5    # 0.125
EPS = 1e-5

f32 = mybir.dt.float32
f32r = mybir.dt.float32r
bf16 = mybir.dt.bfloat16
i32 = mybir.dt.int32
AF = mybir.ActivationFunctionType
OP = mybir.AluOpType
BF = ml_dtypes.bfloat16


def build_program():
    nc = bacc.Bacc("TRN2", target_bir_lowering=False, debug=False, num_devices=NCORES)

    # ---- per-core DRAM tensors (values sharded/preprocessed by host) ----
    x_d = nc.dram_tensor("x_loc", [N, IN], bf16, kind="ExternalInput")
    wq_d = nc.dram_tensor("wq_loc", [IN, FH], bf16, kind="ExternalInput")
    wk_d = nc.dram_tensor("wk_loc", [IN, FH], bf16, kind="ExternalInput")
    wv_d = nc.dram_tensor("wv_loc", [IN, FH], bf16, kind="ExternalInput")
    wout_d = nc.dram_tensor("wout_loc", [FH, IN], bf16, kind="ExternalInput")
    wctx_d = nc.dram_tensor("wctx", [CTX_DIM, 2 * D], bf16, kind="ExternalInput")
    cemb_d = nc.dram_tensor("cemb_loc", [M_CTX, CTX_DIM], bf16, kind="ExternalInput")
    nullkv_d = nc.dram_tensor("nullkv", [2, D], bf16, kind="ExternalInput")
    qkb_d = nc.dram_tensor("qk_bias", [4, 128], f32, kind="ExternalInput")
    cvb_d = nc.dram_tensor("cv_bias", [1, FH], bf16, kind="ExternalInput")
    ckvb_d = nc.dram_tensor("ckv_bias", [128], f32, kind="ExternalInput")
    outg_d = nc.dram_tensor("out_g", [IN], bf16, kind="ExternalInput")
    outb_d = nc.dram_tensor("out_b", [IN], bf16, kind="ExternalInput")
    ident_d = nc.dram_tensor("const_ident", [128, 128], bf16, kind="ExternalInput")
    ones_d = nc.dram_tensor("const_ones", [1, 128], bf16, kind="ExternalInput")
    onesf_d = nc.dram_tensor("const_ones_f32", [1, 64], f32, kind="ExternalInput")
    y_out_d = nc.dram_tensor("y_out", [BLK, IN], bf16, kind="ExternalOutput")
    # internal DRAM for the collective (per-block to avoid WAR hazards)
    ypart_d = [nc.dram_tensor(f"y_partial{b}", [BLK, IN], bf16) for b in range(NBLK)]
    yred_d = [nc.dram_tensor(f"y_red{b}", [128, IN], bf16) for b in range(3)] + [
        [nc.dram_tensor(f"y_red3_{a}", [64, IN], bf16) for a in range(2)]]

    with tile.TileContext(nc) as tc:
        _emit(nc, tc, locals())
    nc.compile()
    return nc


def _emit(nc, tc, t):
    from contextlib import ExitStack

    x_d, cemb_d = t["x_d"], t["cemb_d"]
    wq_d, wk_d, wv_d, wout_d, wctx_d = t["wq_d"], t["wk_d"], t["wv_d"], t["wout_d"], t["wctx_d"]
    nullkv_d = t["nullkv_d"]
    qkb_d, cvb_d, ckvb_d = t["qkb_d"], t["cvb_d"], t["ckvb_d"]
    outg_d, outb_d = t["outg_d"], t["outb_d"]
    y_out_d, ypart_d, yred_d = t["y_out_d"], t["ypart_d"], t["yred_d"]
    ident_d, ones_d, onesf_d = t["ident_d"], t["ones_d"], t["onesf_d"]

    with ExitStack() as ctx:
        persist = ctx.enter_context(tc.tile_pool(name="persist", bufs=1))
        stat = ctx.enter_context(tc.tile_pool(name="stat", bufs=4))

        # ---------------- Phase 0: constants & weights ----------------
        ident = persist.tile([128, 128], bf16, name="ident", tag="ident")
        nc.gpsimd.dma_start(ident, ident_d.ap())
        eps_t = persist.tile([128, 1], f32, name="eps", tag="eps")
        nc.vector.memset(eps_t, EPS)
        magic_t = persist.tile([128, 1], i32, name="magic", tag="magic")
        nc.vector.memset(magic_t, 0x5F3759DF)

        # host-precomputed per-partition biases
        qkb_sb = persist.tile([128, 4], f32, name="qkb_sb", tag="qkb_sb")
        nc.gpsimd.dma_start(qkb_sb, qkb_d.ap().rearrange("a p -> p a"))
        ckvb_sb = persist.tile([128, 1], f32, name="ckvb_sb", tag="ckvb_sb")
        nc.gpsimd.dma_start(ckvb_sb, ckvb_d.ap().rearrange("(a p) -> p a", p=128))

        ones_ap = ones_d.ap()
        ones_r = persist.tile([1, 128], bf16, name="ones_r", tag="ones_r")
        nc.gpsimd.dma_start(ones_r, ones_ap)
        ones2 = persist.tile([65, 64], f32r, name="ones2", tag="ones2")
        nc.gpsimd.dma_start(ones2[64:65, :], onesf_d.ap().bitcast(f32r))
        ones_hg = persist.tile([128, HG], bf16, name="ones_hg", tag="ones_hg")
        nc.gpsimd.dma_start(ones_hg, ones_ap[0:1, 0:HG].to_broadcast([128, HG]))

        # null key/value: knull2 rows 0:64 and 64:128 both = null_k (for the two
        # row-packed head positions); nullv2 rows 0 = [null_v | 1].
        knull2 = persist.tile([128, 1], bf16, name="knull2", tag="knull2")
        nk_ap = nullkv_d.ap()[0:1, :].rearrange("a b -> b a")
        nc.gpsimd.dma_start(knull2[0:64, :], nk_ap)
        nc.gpsimd.dma_start(knull2[64:128, :], nk_ap)
        nullv2 = persist.tile([1, 65], bf16, name="nullv2", tag="nullv2")
        nc.gpsimd.dma_start(nullv2[0:1, 0:64], nullkv_d.ap()[1:2, :])
        nc.gpsimd.dma_start(nullv2[0:1, 64:65], ones_ap[0:1, 0:1])

        # Heavy P0 (weights + context projection), emitted AFTER block-0's LN/transpose
        # chains so the first x tiles hit the DMA queue first.
        wq_sb, wk_sb, wv_sb, wctx_sb, wout_sb = [], [], [], [], []
        cv_row = persist.tile([1, FH], bf16, name="cv_row", tag="cv_row")
        nc.gpsimd.dma_start(cv_row, cvb_d.ap())
        ckvT_sb = persist.tile([128, M_CTX], bf16, name="ckvT", tag="ckvT")
        ck2 = persist.tile([128, M_CTX], bf16, name="ck2", tag="ck2")
        cv_ext = persist.tile([128, 65], bf16, name="cv_ext", tag="cv_ext")

        def emit_p0_heavy(p0sb, psT):
            for name, dram, lst in (("wq", wq_d, wq_sb), ("wk", wk_d, wk_sb), ("wv", wv_d, wv_sb)):
                for c in range(8):
                    w = persist.tile([128, FH], bf16, name=f"{name}{c}", tag=f"{name}{c}")
                    nc.gpsimd.dma_start(w, dram.ap()[128 * c : 128 * (c + 1), :])
                    lst.append(w)
            for c in range(6):
                w = persist.tile([128, 2 * D], bf16, name=f"wctx{c}", tag=f"wctx{c}")
                nc.gpsimd.dma_start(w, wctx_d.ap()[128 * c : 128 * (c + 1), :])
                wctx_sb.append(w)
            for c in range(2):
                w = persist.tile([128, IN], bf16, name=f"wout{c}", tag=f"wout{c}")
                nc.gpsimd.dma_start(w, wout_d.ap()[128 * c : 128 * (c + 1), :])
                wout_sb.append(w)
            # ---- context projection: ckv^T = W_ctx'.T @ LN(c_emb).T + bias ----
            cemb_sb = p0sb.tile([128, CTX_DIM], bf16, name="cemb", tag="cemb")
            nc.gpsimd.dma_start(cemb_sb, cemb_d.ap())
            stc = stat.tile([128, 3, 6], f32, name="stc", tag="stc")
            for i in range(3):
                nc.vector.bn_stats(stc[:, i, :], cemb_sb[:, 256 * i : 256 * (i + 1)])
            mvc = stat.tile([128, 2], f32, name="mvc", tag="mvc")
            nc.vector.bn_aggr(mvc, stc)
            stdc = stat.tile([128, 1], f32, name="stdc", tag="stdc")
            nc.scalar.activation(stdc, mvc[:, 1:2], AF.Sqrt, bias=eps_t[:, 0:1])
            rstd_c = stat.tile([128, 1], f32, name="rstd_c", tag="rstd_c")
            nc.vector.reciprocal_approx_fast(rstd_c, stdc)
            zc = p0sb.tile([128, CTX_DIM], bf16, name="zc", tag="zc")
            nc.vector.tensor_scalar(zc, cemb_sb, mvc[:, 0:1], rstd_c, op0=OP.subtract, op1=OP.mult)
            tpc = psT.tile([128, CTX_DIM], bf16, name="tpc", tag="tp")
            for c in range(6):
                nc.tensor.transpose(tpc[:, 128 * c : 128 * (c + 1)], zc[:, 128 * c : 128 * (c + 1)], ident)
            zcT = p0sb.tile([128, 6, 128], bf16, name="zcT", tag="zcT")
            nc.any.tensor_copy(zcT, tpc.rearrange("p (c w) -> p c w", c=6))
            psk = psT.tile([128, M_CTX], f32, name="psk", tag="tp")
            for c in range(6):
                nc.tensor.matmul(psk, wctx_sb[c], zcT[:, c, :], start=(c == 0), stop=(c == 5))
            nc.vector.tensor_scalar_add(ckvT_sb, psk, ckvb_sb[:, 0:1])
            # ck duplicated into both row-halves (for 2-head row packing)
            nc.gpsimd.dma_start(ck2[0:64, :], ckvT_sb[0:64, :])
            nc.gpsimd.dma_start(ck2[64:128, :], ckvT_sb[0:64, :])
            # cv in normal layout [M_CTX, 64] with a ones column -> [128, 65]
            cvT_tmp = p0sb.tile([64, M_CTX], bf16, name="cvT_tmp", tag="cvT_tmp")
            nc.gpsimd.dma_start(cvT_tmp, ckvT_sb[64:128, :])
            ps_cv = psT.tile([128, 64], bf16, name="ps_cv", tag="tp")
            nc.tensor.transpose(ps_cv, cvT_tmp, ident[0:64, 0:64])
            nc.any.tensor_copy(cv_ext[:, 0:64], ps_cv)
            nc.vector.tensor_copy(cv_ext[:, 64:65], ones_hg[:, 0:1])

        # ---------------- persistent activation tensors ----------------
        qT = [persist.tile([128, N], bf16, name=f"qT{j}", tag=f"qT{j}") for j in range(2)]
        kT = [persist.tile([128, N], bf16, name=f"kT{j}", tag=f"kT{j}") for j in range(2)]
        attnT = [persist.tile([128, N], bf16, name=f"attnT{j}", tag=f"attnT{j}") for j in range(2)]
        v_tiles = []
        for i in range(16):
            vt = persist.tile([128, HG, 65], bf16, name=f"v{i}", tag=f"v{i}")
            nc.vector.tensor_copy(vt[:, :, 64:65], ones_hg.unsqueeze(2))
            v_tiles.append(vt)

        # ---------------- Phase 1: LN(x), transpose, q/k/v projections ----------------
        with tc.tile_pool(name="xp", bufs=3) as xp, \
             tc.tile_pool(name="zp", bufs=2) as zp, \
             tc.tile_pool(name="ztp", bufs=2) as ztp, \
             tc.tile_pool(name="p0sb", bufs=2) as p0sb, \
             tc.tile_pool(name="tpp", bufs=2, space="PSUM") as tpp, \
             tc.tile_pool(name="projp", bufs=2, space="PSUM") as projp, \
             tc.tile_pool(name="vpp", bufs=2, space="PSUM") as vpp:

            def emit_tts(blk):
                zT = ztp.tile([128, 8, BLK], bf16, name="zT", tag="zT")
                for tt in range(4):
                    t0 = BLK * blk + 128 * tt
                    x_t = xp.tile([128, IN], bf16, name="x_t", tag="x_t")
                    nc.sync.dma_start(x_t, x_d.ap()[t0 : t0 + 128, :])
                    st = stat.tile([128, 2, 6], f32, name="st", tag="st")
                    nc.vector.bn_stats(st[:, 0, :], x_t[:, 0:512])
                    nc.vector.bn_stats(st[:, 1, :], x_t[:, 512:1024])
                    mv = stat.tile([128, 2], f32, name="mv", tag="mv")
                    nc.vector.bn_aggr(mv, st)
                    sd = stat.tile([128, 1], f32, name="sd", tag="sd")
                    nc.scalar.activation(sd, mv[:, 1:2], AF.Sqrt, bias=eps_t[:, 0:1])
                    rstd = stat.tile([128, 1], f32, name="rstd", tag="rstd")
                    nc.vector.reciprocal_approx_fast(rstd, sd)
                    z_t = zp.tile([128, IN], bf16, name="z_t", tag="z_t")
                    nc.any.tensor_scalar(z_t, x_t, mv[:, 0:1], rstd, op0=OP.subtract, op1=OP.mult)
                    tp = tpp.tile([128, 1024], bf16, name="tp", tag="tp")
                    for c in range(8):
                        nc.tensor.transpose(tp[:, 128 * c : 128 * (c + 1)], z_t[:, 128 * c : 128 * (c + 1)], ident)
                    nc.scalar.activation(zT[:, :, 128 * tt : 128 * (tt + 1)],
                                         tp.rearrange("p (c w) -> p c w", c=8), AF.Copy)
                return zT

            def emit_proj(blk, zT):
                # q/k projections (transposed layout), per head-pair j
                for wi, (wsb, dst) in enumerate(((wq_sb, qT), (wk_sb, kT))):
                    for j in range(2):
                        ps = projp.tile([128, BLK], f32, name="proj", tag="proj")
                        for c in range(8):
                            nc.tensor.matmul(ps, wsb[c][:, 128 * j : 128 * (j + 1)], zT[:, c, :],
                                             start=(c == 0), stop=(c == 7))
                        nc.scalar.activation(dst[j][:, BLK * blk : BLK * (blk + 1)], ps, AF.Identity,
                                             bias=qkb_sb[:, 2 * wi + j : 2 * wi + j + 1])
                # v projection (normal layout) per 128-token tile
                for tt in range(4):
                    psv = vpp.tile([128, FH], f32, name="psv", tag="psv")
                    for c in range(8):
                        nc.tensor.matmul(psv, zT[:, c, 128 * tt : 128 * (tt + 1)], wv_sb[c],
                                         start=(c == 0), stop=False)
                    nc.tensor.matmul(psv, ones_r, cv_row, start=False, stop=True)
                    vt = v_tiles[4 * blk + tt]
                    nc.any.tensor_copy(vt[:, :, 0:64], psv.rearrange("p (h d) -> p h d", h=HG))

            zT0 = emit_tts(0)
            emit_p0_heavy(p0sb, tpp)
            emit_proj(0, zT0)
            for blk in range(1, NBLK):
                zTb = emit_tts(blk)
                emit_proj(blk, zTb)

        # ---------------- Phases 2-4: attention, out-proj, chunked RS + final LN ----------------
        gout_rep = persist.tile([128, IN], bf16, name="gout_rep", tag="gout_rep")
        nc.gpsimd.dma_start(gout_rep, outg_d.ap().unsqueeze(0).to_broadcast([128, IN]))
        bout_rep = persist.tile([128, IN], bf16, name="bout_rep", tag="bout_rep")
        nc.gpsimd.dma_start(bout_rep, outb_d.ap().unsqueeze(0).to_broadcast([128, IN]))
        with tc.tile_pool(name="wtp", bufs=2) as wtp, \
             tc.tile_pool(name="oddp", bufs=2) as oddp, \
             tc.tile_pool(name="rcpp", bufs=2) as rcpp, \
             tc.tile_pool(name="expnp", bufs=2) as expnp, \
             tc.tile_pool(name="ysb", bufs=3) as ysbp, \
             tc.tile_pool(name="agp", bufs=2) as agp, \
             tc.tile_pool(name="fin", bufs=2) as fin, \
             tc.tile_pool(name="s0p", bufs=3, space="PSUM") as s0p, \
             tc.tile_pool(name="pvp", bufs=2, space="PSUM") as pvp:
            deferred = []
            deferred_fin = []

            def make_final_ln(blk, a, rows):
                # final LN on `rows` received token rows.  rstd via a
                # Quake-seeded Newton rsqrt on DVE (no scalar-engine table
                # swap mid-Exp); normalize chain in bf16 for DVE 2x mode.
                src_d = yred_d[blk] if a is None else yred_d[blk][a]
                row0 = 128 * blk + (0 if a is None else 64 * a)
                def final_ln():
                    yr = fin.tile([rows, IN], bf16, name="yr", tag="yr", bufs=4)
                    nc.gpsimd.dma_start(yr, src_d.ap())
                    st = stat.tile([rows, 2, 6], f32, name="st", tag="st")
                    nc.vector.bn_stats(st[:, 0, :], yr[:, 0:512])
                    nc.vector.bn_stats(st[:, 1, :], yr[:, 512:1024])
                    mv = stat.tile([rows, 2], f32, name="mv", tag="mv")
                    nc.vector.bn_aggr(mv, st)
                    ve = stat.tile([rows, 1], f32, name="ve", tag="ve")
                    nc.vector.tensor_scalar_add(ve, mv[:, 1:2], EPS)
                    t1 = stat.tile([rows, 1], i32, name="t1", tag="t1")
                    nc.vector.tensor_scalar(t1, ve.bitcast(i32), 1, None, op0=OP.arith_shift_right)
                    rstd = fin.tile([rows, 1], f32, name="rstd", tag="rstd", bufs=4)
                    nc.vector.tensor_tensor(rstd.bitcast(i32), magic_t[0:rows, :], t1, op=OP.subtract)
                    nr = stat.tile([rows, 1], f32, name="nr", tag="nr")
                    for _ in range(2):
                        nc.vector.tensor_tensor(nr, rstd, rstd, op=OP.mult)
                        nc.vector.tensor_tensor(nr, nr, ve, op=OP.mult)
                        nc.vector.tensor_scalar(nr, nr, -0.5, 1.5, op0=OP.mult, op1=OP.add)
                        nc.vector.tensor_tensor(rstd, rstd, nr, op=OP.mult)
                    zf = fin.tile([rows, IN], bf16, name="zf", tag="zf", bufs=4)
                    nc.vector.tensor_scalar(zf, yr, mv[:, 0:1], rstd, op0=OP.subtract, op1=OP.mult)
                    nc.vector.tensor_tensor(zf, zf, gout_rep[0:rows, :], op=OP.mult)
                    nc.vector.tensor_tensor(zf, zf, bout_rep[0:rows, :], op=OP.add)
                    nc.gpsimd.dma_start(y_out_d.ap()[row0 : row0 + rows, :], zf)
                return final_ln

            for blk in range(NBLK):
                bsl = slice(BLK * blk, BLK * (blk + 1))
                for pj in range(2):
                    q0 = qT[pj][0:64, bsl]
                    q1 = qT[pj][64:128, bsl]
                    # null-key scores for both heads -> one psum row, one exp
                    expn = expnp.tile([1, 2 * BLK], bf16, name="expn", tag="expn")
                    ps_nl = s0p.tile([1, 2 * BLK], f32, name="ps_nl", tag="ps_s")
                    nc.tensor.matmul(ps_nl[0:1, 0:BLK], knull2[0:64, :], q0, start=True, stop=True)
                    nc.tensor.matmul(ps_nl[0:1, BLK : 2 * BLK], knull2[64:128, :], q1, start=True,
                                     stop=True, tile_position=(64, 0))
                    nc.scalar.activation(expn, ps_nl, AF.Exp, scale=SCALE)
                    # scores -> exp -> PV, pipelined per key tile; both heads share one
                    # [128,1024] scores psum + one exp op (h0 cols 0:512, h1 cols 512:1024).
                    # PV trails one key tile behind so PE never head-of-line blocks on exp.
                    ps_pv0 = pvp.tile([65, BLK], f32, name="ps_pv0", tag="ps_pv")
                    ps_pv1 = pvp.tile([65, BLK], f32, name="ps_pv1", tag="ps_pv")

                    def pv_step(kt, wt):
                        lv0 = cv_ext[:, 0:65] if kt == 16 else v_tiles[kt][:, 2 * pj, :]
                        lv1 = cv_ext[:, 0:65] if kt == 16 else v_tiles[kt][:, 2 * pj + 1, :]
                        nc.tensor.matmul(ps_pv0, lv0, wt[:, 0:BLK], start=(kt == 0), stop=False)
                        nc.tensor.matmul(ps_pv1, lv1, wt[:, BLK : 2 * BLK], start=(kt == 0), stop=False)

                    pending = []
                    for kt in range(KT):
                        if kt == 2 and deferred:
                            deferred.pop(0)()
                        if blk == 3 and pj == 1 and kt == 12 and deferred_fin:
                            deferred_fin.pop(0)()
                        ps_s = s0p.tile([128, 2 * BLK], f32, name="ps_s", tag="ps_s")
                        wt = wtp.tile([128, 2 * BLK], bf16, name="wt", tag="wt", bufs=5)
                        l0 = ck2[0:64, :] if kt == 16 else kT[pj][0:64, 128 * kt : 128 * (kt + 1)]
                        l1 = ck2[64:128, :] if kt == 16 else kT[pj][64:128, 128 * kt : 128 * (kt + 1)]
                        nc.tensor.matmul(ps_s[:, 0:BLK], l0, q0, start=True, stop=True)
                        nc.tensor.matmul(ps_s[:, BLK : 2 * BLK], l1, q1, start=True, stop=True,
                                         tile_position=(64, 0))
                        if len(pending) >= 3:
                            pv_step(*pending.pop(0))
                        nc.scalar.activation(wt, ps_s, AF.Exp, scale=SCALE)
                        pending.append((kt, wt))
                    for args in pending:
                        pv_step(*args)
                    nc.tensor.matmul(ps_pv0, nullv2[0:1, :], expn[0:1, 0:BLK], start=False, stop=True)
                    nc.tensor.matmul(ps_pv1, nullv2[0:1, :], expn[0:1, BLK : 2 * BLK], start=False, stop=True)

                    # normalize: attnT = pv[0:64] * broadcast(1/denominator).  The recip
                    # (DVE) is emitted now so it overlaps the next pair's scores; the PE
                    # broadcast + multiply are deferred into the next pair's kt loop so
                    # the PE stream never head-of-line blocks on the DVE chain.
                    rcps = []
                    for h, ps_pv in ((0, ps_pv0), (1, ps_pv1)):
                        rcp = rcpp.tile([65, BLK], f32r, name="rcp", tag="rcp")
                        with nc.allow_low_precision(reason="fp32r recip of softmax denom"):
                            nc.vector.reciprocal(rcp[64:65, :], ps_pv[64:65, :])
                        rcps.append(rcp)

                    def do_norm(pj=pj, bsl=bsl, pvs=(ps_pv0, ps_pv1), rcps=tuple(rcps)):
                        for h, (ps_pv, rcp) in enumerate(zip(pvs, rcps)):
                            ps_rb = s0p.tile([64, BLK], f32, name="ps_rb", tag="ps_s")
                            nc.tensor.matmul(ps_rb, ones2[64:65, :], rcp[64:65, :],
                                             start=True, stop=True, tile_position=(64, 0))
                            rb_sb = rcpp.tile([64, BLK], f32, name="rb_sb", tag="rb_sb")
                            nc.vector.tensor_copy(rb_sb, ps_rb)
                            if h == 0:
                                nc.vector.tensor_tensor(attnT[pj][0:64, bsl], ps_pv[0:64, :], rb_sb, op=OP.mult)
                            else:
                                tmp = oddp.tile([64, BLK], bf16, name="odd", tag="odd")
                                nc.vector.tensor_tensor(tmp, ps_pv[0:64, :], rb_sb, op=OP.mult)
                                nc.sync.dma_start(attnT[pj][64:128, bsl], tmp)

                    deferred.append(do_norm)
                # flush pending normalizations, then out-projection for this block
                while deferred:
                    deferred.pop(0)()
                for tt4 in range(4):
                    tt = 4 * blk + tt4
                    y_sb = ysbp.tile([128, IN], bf16, name="y_sb", tag="y_sb")
                    for nh in range(2):
                        ps_y = s0p.tile([128, 512], f32, name="ps_y", tag="ps_s")
                        for c in range(2):
                            nc.tensor.matmul(ps_y, attnT[c][:, 128 * tt : 128 * (tt + 1)],
                                             wout_sb[c][:, 512 * nh : 512 * (nh + 1)],
                                             start=(c == 0), stop=(c == 1))
                        nc.vector.tensor_copy(y_sb[:, 512 * nh : 512 * (nh + 1)], ps_y)
                    nc.sync.dma_start(ypart_d[blk].ap()[128 * tt4 : 128 * (tt4 + 1), :], y_sb)
                    # blocks 0-2: one ReduceScatter per block; block 3: two
                    # half RS so the tail only waits on the last 256 rows
                    if blk < 3 and tt4 == 3:
                        nc.gpsimd.collective_compute(
                            "ReduceScatter",
                            OP.add,
                            replica_groups=[[0, 1, 2, 3], [4, 5, 6, 7]],
                            ins=[ypart_d[blk].ap()],
                            outs=[yred_d[blk].ap()],
                        )
                        deferred_fin.append(make_final_ln(blk, None, 128))
                    elif blk == 3 and tt4 in (1, 3):
                        a = tt4 // 2
                        nc.gpsimd.collective_compute(
                            "ReduceScatter",
                            OP.add,
                            replica_groups=[[0, 1, 2, 3], [4, 5, 6, 7]],
                            ins=[ypart_d[blk].ap()[256 * a : 256 * (a + 1), :]],
                            outs=[yred_d[blk][a].ap()],
                        )
                        deferred_fin.append(make_final_ln(blk, a, 64))
            while deferred_fin:
                deferred_fin.pop(0)()


def shard_inputs(inputs):
    """Split full inputs into 8 per-core input maps (host-side LN-gamma folding,
    bias precompute, bf16 casts)."""
    f = lambda v: np.asarray(v, np.float32)
    x = f(inputs["x"])
    c_emb = f(inputs["c_emb"])
    ln_g, ln_b = f(inputs["ln_g"]), f(inputs["ln_b"])
    ctx_g, ctx_b = f(inputs["ctx_ln_g"]), f(inputs["ctx_ln_b"])
    W_q = (ln_g[:, None] * f(inputs["W_q"])).reshape(IN, H, D)
    W_kv = (ln_g[:, None] * f(inputs["W_kv"])).reshape(IN, 2, H, D)
    W_ctx = ctx_g[:, None] * f(inputs["W_ctx"])
    W_out = f(inputs["W_out"]).reshape(H, D, IN)
    q_bias = (ln_b @ W_q.reshape(IN, H * D)).reshape(H, D)
    kv_bias = (ln_b @ W_kv.reshape(IN, 2 * H * D)).reshape(2, H, D)
    ckv_bias = ctx_b @ W_ctx + f(inputs["b_ctx"])
    common = {
        "const_ident": np.eye(128, dtype=BF),
        "const_ones": np.ones((1, 128), BF),
        "const_ones_f32": np.ones((1, 64), np.float32),
        "wctx": np.ascontiguousarray(W_ctx.astype(BF)),
        "nullkv": f(inputs["null_kv"]).astype(BF),
        "ckv_bias": np.ascontiguousarray(ckv_bias, dtype=np.float32),
        "out_g": f(inputs["out_ln_g"]).astype(BF),
        "out_b": f(inputs["out_ln_b"]).astype(BF),
    }
    in_maps = []
    for c in range(NCORES):
        b, g = c // 4, c % 4
        hs = slice(HG * g, HG * (g + 1))
        qkb = np.stack([q_bias[hs].reshape(FH)[0:128], q_bias[hs].reshape(FH)[128:256],
                        kv_bias[0, hs].reshape(FH)[0:128], kv_bias[0, hs].reshape(FH)[128:256]])
        in_maps.append({
            "x_loc": x[b].astype(BF),
            "cemb_loc": c_emb[b].astype(BF),
            "wq_loc": np.ascontiguousarray(W_q[:, hs].reshape(IN, FH).astype(BF)),
            "wk_loc": np.ascontiguousarray(W_kv[:, 0, hs].reshape(IN, FH).astype(BF)),
            "wv_loc": np.ascontiguousarray(W_kv[:, 1, hs].reshape(IN, FH).astype(BF)),
            "wout_loc": np.ascontiguousarray(W_out[hs].reshape(FH, IN).astype(BF)),
            "qk_bias": np.ascontiguousarray(qkb, dtype=np.float32),
            "cv_bias": np.ascontiguousarray(kv_bias[1, hs].reshape(1, FH).astype(BF)),
            **common,
        })
    return in_maps


def unshard(results):
    out = np.empty((B, N, IN), np.float32)
    for c in range(NCORES):
        b, r = c // 4, c % 4
        y = np.asarray(results[c]["y_out"], dtype=np.float32)
        for blk in range(3):
            t0 = BLK * blk + 128 * r
            out[b, t0 : t0 + 128, :] = y[128 * blk : 128 * (blk + 1)]
        for a in range(2):
            t0 = BLK * 3 + 256 * a + 64 * r
            y0 = 384 + 64 * a
            out[b, t0 : t0 + 64, :] = y[y0 : y0 + 64]
    return out


_CACHE = {}


def kernel(**inputs) -> np.ndarray:
    from concourse.bass_utils import run_bass_kernel_spmd

    if "nc" not in _CACHE:
        _CACHE["nc"] = build_program()
    nc = _CACHE["nc"]
    in_maps = shard_inputs(inputs)
    res = run_bass_kernel_spmd(nc, in_maps, list(range(NCORES))).results
    return unshard(res)


if __name__ == "__main__":
    nc = build_program()
    print("program built OK;",
          sum(1 for _ in nc.inst_map), "instructions")


# revision 18
# speedup vs baseline: 1.0439x; 1.0154x over previous
"""Trainium2 Bass kernel for nn_MultiHeadAttention_81999515616076.

Reference computation (per batch b):
    xn = LN(x)                                    [N, IN]
    q  = xn @ W_q   -> [N, H, D]
    k,v= xn @ W_kv  -> [N, H, D] each
    ckv= LN(c_emb) @ W_ctx + b_ctx -> ck, cv      [M, D] (shared across heads)
    keys per head = [self keys (N)] + [null key] + [ctx keys (M)]  (2177 total)
    out = softmax(q.k / sqrt(D)) @ values         [N, H, D]
    y  = LN(out.reshape(N, H*D) @ W_out)          [N, IN]

Sharding (8 cores): core c -> batch b = c//4, head group g = c%4 (heads 4g..4g+3).
Per-core: LN+transpose of x, bf16 projections, flash-style attention for its 4
heads (scores computed transposed: [keys, tokens]; softmax denominator via a
ones-column in the PV matmul; no max subtraction -- scores are bounded ~N(0,0.4)),
out-projection partials, bf16 ReduceScatter(add) over the 4 cores of each batch,
and final LN on the received 512-token slice.  Host folds the input-LN gamma into
the projection weights, precomputes all LN-beta biases, and casts weights +
activations to bf16 (fp32 statistics / PSUM accumulation on device).
"""

import sys

sys.path.insert(0, "/opt/trn_rl_repo")

import numpy as np
import ml_dtypes

import concourse.bacc as bacc
import concourse.tile as tile
import concourse.mybir as mybir

B, N, IN = 2, 2048, 1024
H, D = 16, 64
CTX_DIM, M_CTX = 768, 128
NCORES = 8
HG = 4               # heads per core
FH = HG * D          # 256 local head-feats
BLK = 512            # token block
NBLK = N // BLK      # 4
KT = 17              # 16 self key tiles + 1 ctx key tile (null key handled separately)
SCALE = D ** -0.5    # 0.125
EPS = 1e-5

f32 = mybir.dt.float32
f32r = mybir.dt.float32r
bf16 = mybir.dt.bfloat16
i32 = mybir.dt.int32
AF = mybir.ActivationFunctionType
OP = mybir.AluOpType
BF = ml_dtypes.bfloat16


def build_program():
    nc = bacc.Bacc("TRN2", target_bir_lowering=False, debug=False, num_devices=NCORES)

    # ---- per-core DRAM tensors (values sharded/preprocessed by host) ----
    x_d = nc.dram_tensor("x_loc", [N, IN], bf16, kind="ExternalInput")
    wq_d = nc.dram_tensor("wq_loc", [IN, FH], bf16, kind="ExternalInput")
    wk_d = nc.dram_tensor("wk_loc", [IN, FH], bf16, kind="ExternalInput")
    wv_d = nc.dram_tensor("wv_loc", [IN, FH], bf16, kind="ExternalInput")
    wout_d = nc.dram_tensor("wout_loc", [FH, IN], bf16, kind="ExternalInput")
    wctx_d = nc.dram_tensor("wctx", [CTX_DIM, 2 * D], bf16, kind="ExternalInput")
    cemb_d = nc.dram_tensor("cemb_loc", [M_CTX, CTX_DIM], bf16, kind="ExternalInput")
    nullkv_d = nc.dram_tensor("nullkv", [2, D], bf16, kind="ExternalInput")
    qkb_d = nc.dram_tensor("qk_bias", [4, 128], f32, kind="ExternalInput")
    cvb_d = nc.dram_tensor("cv_bias", [1, FH], bf16, kind="ExternalInput")
    ckvb_d = nc.dram_tensor("ckv_bias", [128], f32, kind="ExternalInput")
    outg_d = nc.dram_tensor("out_g", [IN], bf16, kind="ExternalInput")
    outb_d = nc.dram_tensor("out_b", [IN], bf16, kind="ExternalInput")
    ident_d = nc.dram_tensor("const_ident", [128, 128], bf16, kind="ExternalInput")
    ones_d = nc.dram_tensor("const_ones", [1, 128], bf16, kind="ExternalInput")
    onesf_d = nc.dram_tensor("const_ones_f32", [1, 64], f32, kind="ExternalInput")
    y_out_d = nc.dram_tensor("y_out", [BLK, IN], bf16, kind="ExternalOutput")
    # internal DRAM for the collective (per-block to avoid WAR hazards)
    ypart_d = [nc.dram_tensor(f"y_partial{b}", [BLK, IN], bf16) for b in range(NBLK)]
    yred_d = [nc.dram_tensor(f"y_red{b}", [128, IN], bf16) for b in range(3)] + [
        [nc.dram_tensor(f"y_red3_{a}", [64, IN], bf16) for a in range(2)]]

    with tile.TileContext(nc) as tc:
        _emit(nc, tc, locals())
    nc.compile()
    return nc


def _emit(nc, tc, t):
    from contextlib import ExitStack

    x_d, cemb_d = t["x_d"], t["cemb_d"]
    wq_d, wk_d, wv_d, wout_d, wctx_d = t["wq_d"], t["wk_d"], t["wv_d"], t["wout_d"], t["wctx_d"]
    nullkv_d = t["nullkv_d"]
    qkb_d, cvb_d, ckvb_d = t["qkb_d"], t["cvb_d"], t["ckvb_d"]
    outg_d, outb_d = t["outg_d"], t["outb_d"]
    y_out_d, ypart_d, yred_d = t["y_out_d"], t["ypart_d"], t["yred_d"]
    ident_d, ones_d, onesf_d = t["ident_d"], t["ones_d"], t["onesf_d"]

    with ExitStack() as ctx:
        persist = ctx.enter_context(tc.tile_pool(name="persist", bufs=1))
        stat = ctx.enter_context(tc.tile_pool(name="stat", bufs=4))

        # ---------------- Phase 0: constants & weights ----------------
        ident = persist.tile([128, 128], bf16, name="ident", tag="ident")
        nc.gpsimd.dma_start(ident, ident_d.ap())
        eps_t = persist.tile([128, 1], f32, name="eps", tag="eps")
        nc.vector.memset(eps_t, EPS)
        magic_t = persist.tile([128, 1], i32, name="magic", tag="magic")
        nc.vector.memset(magic_t, 0x5F3759DF)

        # host-precomputed per-partition biases
        qkb_sb = persist.tile([128, 4], f32, name="qkb_sb", tag="qkb_sb")
        nc.gpsimd.dma_start(qkb_sb, qkb_d.ap().rearrange("a p -> p a"))
        ckvb_sb = persist.tile([128, 1], f32, name="ckvb_sb", tag="ckvb_sb")
        nc.gpsimd.dma_start(ckvb_sb, ckvb_d.ap().rearrange("(a p) -> p a", p=128))

        ones_ap = ones_d.ap()
        ones_r = persist.tile([1, 128], bf16, name="ones_r", tag="ones_r")
        nc.gpsimd.dma_start(ones_r, ones_ap)
        ones2 = persist.tile([65, 64], f32r, name="ones2", tag="ones2")
        nc.gpsimd.dma_start(ones2[64:65, :], onesf_d.ap().bitcast(f32r))
        ones_hg = persist.tile([128, HG], bf16, name="ones_hg", tag="ones_hg")
        nc.gpsimd.dma_start(ones_hg, ones_ap[0:1, 0:HG].to_broadcast([128, HG]))

        # null key/value: knull2 rows 0:64 and 64:128 both = null_k (for the two
        # row-packed head positions); nullv2 rows 0 = [null_v | 1].
        knull2 = persist.tile([128, 1], bf16, name="knull2", tag="knull2")
        nk_ap = nullkv_d.ap()[0:1, :].rearrange("a b -> b a")
        nc.gpsimd.dma_start(knull2[0:64, :], nk_ap)
        nc.gpsimd.dma_start(knull2[64:128, :], nk_ap)
        nullv2 = persist.tile([1, 65], bf16, name="nullv2", tag="nullv2")
        nc.gpsimd.dma_start(nullv2[0:1, 0:64], nullkv_d.ap()[1:2, :])
        nc.gpsimd.dma_start(nullv2[0:1, 64:65], ones_ap[0:1, 0:1])

        # Heavy P0 (weights + context projection), emitted AFTER block-0's LN/transpose
        # chains so the first x tiles hit the DMA queue first.
        wq_sb, wk_sb, wv_sb, wctx_sb, wout_sb = [], [], [], [], []
        cv_row = persist.tile([1, FH], bf16, name="cv_row", tag="cv_row")
        nc.gpsimd.dma_start(cv_row, cvb_d.ap())
        ckvT_sb = persist.tile([128, M_CTX], bf16, name="ckvT", tag="ckvT")
        ck2 = persist.tile([128, M_CTX], bf16, name="ck2", tag="ck2")
        cv_ext = persist.tile([128, 65], bf16, name="cv_ext", tag="cv_ext")

        def emit_p0_heavy(p0sb, psT):
            for name, dram, lst in (("wq", wq_d, wq_sb), ("wk", wk_d, wk_sb), ("wv", wv_d, wv_sb)):
                for c in range(8):
                    w = persist.tile([128, FH], bf16, name=f"{name}{c}", tag=f"{name}{c}")
                    nc.gpsimd.dma_start(w, dram.ap()[128 * c : 128 * (c + 1), :])
                    lst.append(w)
            for c in range(6):
                w = persist.tile([128, 2 * D], bf16, name=f"wctx{c}", tag=f"wctx{c}")
                nc.gpsimd.dma_start(w, wctx_d.ap()[128 * c : 128 * (c + 1), :])
                wctx_sb.append(w)
            for c in range(2):
                w = persist.tile([128, IN], bf16, name=f"wout{c}", tag=f"wout{c}")
                nc.gpsimd.dma_start(w, wout_d.ap()[128 * c : 128 * (c + 1), :])
                wout_sb.append(w)
            # ---- context projection: ckv^T = W_ctx'.T @ LN(c_emb).T + bias ----
            cemb_sb = p0sb.tile([128, CTX_DIM], bf16, name="cemb", tag="cemb")
            nc.gpsimd.dma_start(cemb_sb, cemb_d.ap())
            stc = stat.tile([128, 3, 6], f32, name="stc", tag="stc")
            for i in range(3):
                nc.vector.bn_stats(stc[:, i, :], cemb_sb[:, 256 * i : 256 * (i + 1)])
            mvc = stat.tile([128, 2], f32, name="mvc", tag="mvc")
            nc.vector.bn_aggr(mvc, stc)
            stdc = stat.tile([128, 1], f32, name="stdc", tag="stdc")
            nc.scalar.activation(stdc, mvc[:, 1:2], AF.Sqrt, bias=eps_t[:, 0:1])
            rstd_c = stat.tile([128, 1], f32, name="rstd_c", tag="rstd_c")
            nc.vector.reciprocal_approx_fast(rstd_c, stdc)
            zc = p0sb.tile([128, CTX_DIM], bf16, name="zc", tag="zc")
            nc.vector.tensor_scalar(zc, cemb_sb, mvc[:, 0:1], rstd_c, op0=OP.subtract, op1=OP.mult)
            tpc = psT.tile([128, CTX_DIM], bf16, name="tpc", tag="tp")
            for c in range(6):
                nc.tensor.transpose(tpc[:, 128 * c : 128 * (c + 1)], zc[:, 128 * c : 128 * (c + 1)], ident)
            zcT = p0sb.tile([128, 6, 128], bf16, name="zcT", tag="zcT")
            nc.any.tensor_copy(zcT, tpc.rearrange("p (c w) -> p c w", c=6))
            psk = psT.tile([128, M_CTX], f32, name="psk", tag="tp")
            for c in range(6):
                nc.tensor.matmul(psk, wctx_sb[c], zcT[:, c, :], start=(c == 0), stop=(c == 5))
            nc.vector.tensor_scalar_add(ckvT_sb, psk, ckvb_sb[:, 0:1])
            # ck duplicated into both row-halves (for 2-head row packing)
            nc.gpsimd.dma_start(ck2[0:64, :], ckvT_sb[0:64, :])
            nc.gpsimd.dma_start(ck2[64:128, :], ckvT_sb[0:64, :])
            # cv in normal layout [M_CTX, 64] with a ones column -> [128, 65]
            cvT_tmp = p0sb.tile([64, M_CTX], bf16, name="cvT_tmp", tag="cvT_tmp")
            nc.gpsimd.dma_start(cvT_tmp, ckvT_sb[64:128, :])
            ps_cv = psT.tile([128, 64], bf16, name="ps_cv", tag="tp")
            nc.tensor.transpose(ps_cv, cvT_tmp, ident[0:64, 0:64])
            nc.any.tensor_copy(cv_ext[:, 0:64], ps_cv)
            nc.vector.tensor_copy(cv_ext[:, 64:65], ones_hg[:, 0:1])

        # ---------------- persistent activation tensors ----------------
        qT = [persist.tile([128, N], bf16, name=f"qT{j}", tag=f"qT{j}") for j in range(2)]
        kT = [persist.tile([128, N], bf16, name=f"kT{j}", tag=f"kT{j}") for j in range(2)]
        attnT = [persist.tile([128, N], bf16, name=f"attnT{j}", tag=f"attnT{j}") for j in range(2)]
        v_tiles = []
        for i in range(16):
            vt = persist.tile([128, HG, 65], bf16, name=f"v{i}", tag=f"v{i}")
            nc.vector.tensor_copy(vt[:, :, 64:65], ones_hg.unsqueeze(2))
            v_tiles.append(vt)

        # ---------------- Phase 1: LN(x), transpose, q/k/v projections ----------------
        with tc.tile_pool(name="xp", bufs=3) as xp, \
             tc.tile_pool(name="zp", bufs=2) as zp, \
             tc.tile_pool(name="ztp", bufs=2) as ztp, \
             tc.tile_pool(name="p0sb", bufs=2) as p0sb, \
             tc.tile_pool(name="tpp", bufs=2, space="PSUM") as tpp, \
             tc.tile_pool(name="projp", bufs=2, space="PSUM") as projp, \
             tc.tile_pool(name="vpp", bufs=2, space="PSUM") as vpp:

            def emit_tts(blk):
                zT = ztp.tile([128, 8, BLK], bf16, name="zT", tag="zT")
                for tt in range(4):
                    t0 = BLK * blk + 128 * tt
                    x_t = xp.tile([128, IN], bf16, name="x_t", tag="x_t")
                    nc.sync.dma_start(x_t, x_d.ap()[t0 : t0 + 128, :])
                    st = stat.tile([128, 2, 6], f32, name="st", tag="st")
                    nc.vector.bn_stats(st[:, 0, :], x_t[:, 0:512])
                    nc.vector.bn_stats(st[:, 1, :], x_t[:, 512:1024])
                    mv = stat.tile([128, 2], f32, name="mv", tag="mv")
                    nc.vector.bn_aggr(mv, st)
                    sd = stat.tile([128, 1], f32, name="sd", tag="sd")
                    nc.scalar.activation(sd, mv[:, 1:2], AF.Sqrt, bias=eps_t[:, 0:1])
                    rstd = stat.tile([128, 1], f32, name="rstd", tag="rstd")
                    nc.vector.reciprocal_approx_fast(rstd, sd)
                    z_t = zp.tile([128, IN], bf16, name="z_t", tag="z_t")
                    nc.any.tensor_scalar(z_t, x_t, mv[:, 0:1], rstd, op0=OP.subtract, op1=OP.mult)
                    tp = tpp.tile([128, 1024], bf16, name="tp", tag="tp")
                    for c in range(8):
                        nc.tensor.transpose(tp[:, 128 * c : 128 * (c + 1)], z_t[:, 128 * c : 128 * (c + 1)], ident)
                    nc.scalar.activation(zT[:, :, 128 * tt : 128 * (tt + 1)],
                                         tp.rearrange("p (c w) -> p c w", c=8), AF.Copy)
                return zT

            def emit_proj(blk, zT):
                # q/k projections (transposed layout), per head-pair j
                for wi, (wsb, dst) in enumerate(((wq_sb, qT), (wk_sb, kT))):
                    for j in range(2):
                        ps = projp.tile([128, BLK], f32, name="proj", tag="proj")
                        for c in range(8):
                            nc.tensor.matmul(ps, wsb[c][:, 128 * j : 128 * (j + 1)], zT[:, c, :],
                                             start=(c == 0), stop=(c == 7))
                        nc.scalar.activation(dst[j][:, BLK * blk : BLK * (blk + 1)], ps, AF.Identity,
                                             bias=qkb_sb[:, 2 * wi + j : 2 * wi + j + 1])
                # v projection (normal layout) per 128-token tile
                for tt in range(4):
                    psv = vpp.tile([128, FH], f32, name="psv", tag="psv")
                    for c in range(8):
                        nc.tensor.matmul(psv, zT[:, c, 128 * tt : 128 * (tt + 1)], wv_sb[c],
                                         start=(c == 0), stop=False)
                    nc.tensor.matmul(psv, ones_r, cv_row, start=False, stop=True)
                    vt = v_tiles[4 * blk + tt]
                    nc.any.tensor_copy(vt[:, :, 0:64], psv.rearrange("p (h d) -> p h d", h=HG))

            zT0 = emit_tts(0)
            emit_p0_heavy(p0sb, tpp)
            emit_proj(0, zT0)
            for blk in range(1, NBLK):
                zTb = emit_tts(blk)
                emit_proj(blk, zTb)

        # ---------------- Phases 2-4: attention, out-proj, chunked RS + final LN ----------------
        gout_rep = persist.tile([128, IN], bf16, name="gout_rep", tag="gout_rep")
        nc.gpsimd.dma_start(gout_rep, outg_d.ap().unsqueeze(0).to_broadcast([128, IN]))
        bout_rep = persist.tile([128, IN], bf16, name="bout_rep", tag="bout_rep")
        nc.gpsimd.dma_start(bout_rep, outb_d.ap().unsqueeze(0).to_broadcast([128, IN]))
        with tc.tile_pool(name="wtp", bufs=2) as wtp, \
             tc.tile_pool(name="oddp", bufs=2) as oddp, \
             tc.tile_pool(name="rcpp", bufs=2) as rcpp, \
             tc.tile_pool(name="expnp", bufs=2) as expnp, \
             tc.tile_pool(name="ysb", bufs=3) as ysbp, \
             tc.tile_pool(name="agp", bufs=2) as agp, \
             tc.tile_pool(name="fin", bufs=2) as fin, \
             tc.tile_pool(name="s0p", bufs=3, space="PSUM") as s0p, \
             tc.tile_pool(name="pvp", bufs=2, space="PSUM") as pvp:
            deferred = []
            deferred_fin = []

            def make_final_ln(blk, a, rows):
                # final LN on `rows` received token rows.  rstd via a
                # Quake-seeded Newton rsqrt on DVE (no scalar-engine table
                # swap mid-Exp); normalize chain in bf16 for DVE 2x mode.
                src_d = yred_d[blk] if a is None else yred_d[blk][a]
                row0 = 128 * blk + (0 if a is None else 64 * a)
                def final_ln():
                    yr = fin.tile([rows, IN], bf16, name="yr", tag="yr", bufs=4)
                    nc.gpsimd.dma_start(yr, src_d.ap())
                    st = stat.tile([rows, 2, 6], f32, name="st", tag="st")
                    nc.vector.bn_stats(st[:, 0, :], yr[:, 0:512])
                    nc.vector.bn_stats(st[:, 1, :], yr[:, 512:1024])
                    mv = stat.tile([rows, 2], f32, name="mv", tag="mv")
                    nc.vector.bn_aggr(mv, st)
                    ve = stat.tile([rows, 1], f32, name="ve", tag="ve")
                    nc.vector.tensor_scalar_add(ve, mv[:, 1:2], EPS)
                    t1 = stat.tile([rows, 1], i32, name="t1", tag="t1")
                    nc.vector.tensor_scalar(t1, ve.bitcast(i32), 1, None, op0=OP.arith_shift_right)
                    rstd = fin.tile([rows, 1], f32, name="rstd", tag="rstd", bufs=4)
                    nc.vector.tensor_tensor(rstd.bitcast(i32), magic_t[0:rows, :], t1, op=OP.subtract)
                    nr = stat.tile([rows, 1], f32, name="nr", tag="nr")
                    for _ in range(2):
                        nc.vector.tensor_tensor(nr, rstd, rstd, op=OP.mult)
                        nc.vector.tensor_tensor(nr, nr, ve, op=OP.mult)
                        nc.vector.tensor_scalar(nr, nr, -0.5, 1.5, op0=OP.mult, op1=OP.add)
                        nc.vector.tensor_tensor(rstd, rstd, nr, op=OP.mult)
                    zf = fin.tile([rows, IN], bf16, name="zf", tag="zf", bufs=4)
                    nc.vector.tensor_scalar(zf, yr, mv[:, 0:1], rstd, op0=OP.subtract, op1=OP.mult)
                    nc.vector.tensor_tensor(zf, zf, gout_rep[0:rows, :], op=OP.mult)
                    nc.vector.tensor_tensor(zf, zf, bout_rep[0:rows, :], op=OP.add)
                    nc.gpsimd.dma_start(y_out_d.ap()[row0 : row0 + rows, :], zf)
                return final_ln

            for blk in range(NBLK):
                bsl = slice(BLK * blk, BLK * (blk + 1))
                for pj in range(2):
                    q0 = qT[pj][0:64, bsl]
                    q1 = qT[pj][64:128, bsl]
                    # null-key scores for both heads -> one psum row, one exp
                    expn = expnp.tile([1, 2 * BLK], bf16, name="expn", tag="expn")
                    ps_nl = s0p.tile([1, 2 * BLK], f32, name="ps_nl", tag="ps_s")
                    nc.tensor.matmul(ps_nl[0:1, 0:BLK], knull2[0:64, :], q0, start=True, stop=True)
                    nc.tensor.matmul(ps_nl[0:1, BLK : 2 * BLK], knull2[64:128, :], q1, start=True,
                                     stop=True, tile_position=(64, 0))
                    nc.scalar.activation(expn, ps_nl, AF.Exp, scale=SCALE)
                    # scores -> exp -> PV, pipelined per key tile; both heads share one
                    # [128,1024] scores psum + one exp op (h0 cols 0:512, h1 cols 512:1024).
                    # PV trails one key tile behind so PE never head-of-line blocks on exp.
                    ps_pv0 = pvp.tile([65, BLK], f32, name="ps_pv0", tag="ps_pv")
                    ps_pv1 = pvp.tile([65, BLK], f32, name="ps_pv1", tag="ps_pv")

                    def pv_step(kt, wt):
                        lv0 = cv_ext[:, 0:65] if kt == 16 else v_tiles[kt][:, 2 * pj, :]
                        lv1 = cv_ext[:, 0:65] if kt == 16 else v_tiles[kt][:, 2 * pj + 1, :]
                        nc.tensor.matmul(ps_pv0, lv0, wt[:, 0:BLK], start=(kt == 0), stop=False)
                        nc.tensor.matmul(ps_pv1, lv1, wt[:, BLK : 2 * BLK], start=(kt == 0), stop=False)

                    pending = []
                    for kt in range(KT):
                        if kt == 2 and deferred:
                            deferred.pop(0)()
                        if blk == 3 and pj == 1 and kt == 12 and deferred_fin:
                            deferred_fin.pop(0)()
                        ps_s = s0p.tile([128, 2 * BLK], f32, name="ps_s", tag="ps_s")
                        wt = wtp.tile([128, 2 * BLK], bf16, name="wt", tag="wt", bufs=7)
                        l0 = ck2[0:64, :] if kt == 16 else kT[pj][0:64, 128 * kt : 128 * (kt + 1)]
                        l1 = ck2[64:128, :] if kt == 16 else kT[pj][64:128, 128 * kt : 128 * (kt + 1)]
                        nc.tensor.matmul(ps_s[:, 0:BLK], l0, q0, start=True, stop=True)
                        nc.tensor.matmul(ps_s[:, BLK : 2 * BLK], l1, q1, start=True, stop=True,
                                         tile_position=(64, 0))
                        if len(pending) >= 3:
                            pv_step(*pending.pop(0))
                        nc.scalar.activation(wt, ps_s, AF.Exp, scale=SCALE)
                        pending.append((kt, wt))
                    for args in pending:
                        pv_step(*args)
                    nc.tensor.matmul(ps_pv0, nullv2[0:1, :], expn[0:1, 0:BLK], start=False, stop=True)
                    nc.tensor.matmul(ps_pv1, nullv2[0:1, :], expn[0:1, BLK : 2 * BLK], start=False, stop=True)

                    # normalize: attnT = pv[0:64] * broadcast(1/denominator).  The recip
                    # (DVE) is emitted now so it overlaps the next pair's scores; the PE
                    # broadcast + multiply are deferred into the next pair's kt loop so
                    # the PE stream never head-of-line blocks on the DVE chain.
                    rcps = []
                    for h, ps_pv in ((0, ps_pv0), (1, ps_pv1)):
                        rcp = rcpp.tile([65, BLK], f32r, name="rcp", tag="rcp")
                        with nc.allow_low_precision(reason="fp32r recip of softmax denom"):
                            nc.vector.reciprocal(rcp[64:65, :], ps_pv[64:65, :])
                        rcps.append(rcp)

                    def do_norm(pj=pj, bsl=bsl, pvs=(ps_pv0, ps_pv1), rcps=tuple(rcps)):
                        for h, (ps_pv, rcp) in enumerate(zip(pvs, rcps)):
                            ps_rb = s0p.tile([64, BLK], f32, name="ps_rb", tag="ps_s")
                            nc.tensor.matmul(ps_rb, ones2[64:65, :], rcp[64:65, :],
                                             start=True, stop=True, tile_position=(64, 0))
                            rb_sb = rcpp.tile([64, BLK], f32, name="rb_sb", tag="rb_sb")
                            nc.vector.tensor_copy(rb_sb, ps_rb)
                            if h == 0:
                                nc.vector.tensor_tensor(attnT[pj][0:64, bsl], ps_pv[0:64, :], rb_sb, op=OP.mult)
                            else:
                                tmp = oddp.tile([64, BLK], bf16, name="odd", tag="odd")
                                nc.vector.tensor_tensor(tmp, ps_pv[0:64, :], rb_sb, op=OP.mult)
                                nc.sync.dma_start(attnT[pj][64:128, bsl], tmp)

                    deferred.append(do_norm)
                # flush pending normalizations, then out-projection for this block
                while deferred:
                    deferred.pop(0)()
                for tt4 in range(4):
                    tt = 4 * blk + tt4
                    y_sb = ysbp.tile([128, IN], bf16, name="y_sb", tag="y_sb")
                    for nh in range(2):
                        ps_y = pvp.tile([128, 512], f32, name="ps_y", tag="ps_pv")
                        for c in range(2):
                            nc.tensor.matmul(ps_y, attnT[c][:, 128 * tt : 128 * (tt + 1)],
                                             wout_sb[c][:, 512 * nh : 512 * (nh + 1)],
                                             start=(c == 0), stop=(c == 1))
                        nc.vector.tensor_copy(y_sb[:, 512 * nh : 512 * (nh + 1)], ps_y)
                    nc.sync.dma_start(ypart_d[blk].ap()[128 * tt4 : 128 * (tt4 + 1), :], y_sb)
                    # blocks 0-2: one ReduceScatter per block; block 3: two
                    # half RS so the tail only waits on the last 256 rows
                    if blk < 3 and tt4 == 3:
                        nc.gpsimd.collective_compute(
                            "ReduceScatter",
                            OP.add,
                            replica_groups=[[0, 1, 2, 3], [4, 5, 6, 7]],
                            ins=[ypart_d[blk].ap()],
                            outs=[yred_d[blk].ap()],
                        )
                        deferred_fin.append(make_final_ln(blk, None, 128))
                    elif blk == 3 and tt4 in (1, 3):
                        a = tt4 // 2
                        nc.gpsimd.collective_compute(
                            "ReduceScatter",
                            OP.add,
                            replica_groups=[[0, 1, 2, 3], [4, 5, 6, 7]],
                            ins=[ypart_d[blk].ap()[256 * a : 256 * (a + 1), :]],
                            outs=[yred_d[blk][a].ap()],
                        )
                        deferred_fin.append(make_final_ln(blk, a, 64))
            while deferred_fin:
                deferred_fin.pop(0)()


def shard_inputs(inputs):
    """Split full inputs into 8 per-core input maps (host-side LN-gamma folding,
    bias precompute, bf16 casts)."""
    f = lambda v: np.asarray(v, np.float32)
    x = f(inputs["x"])
    c_emb = f(inputs["c_emb"])
    ln_g, ln_b = f(inputs["ln_g"]), f(inputs["ln_b"])
    ctx_g, ctx_b = f(inputs["ctx_ln_g"]), f(inputs["ctx_ln_b"])
    W_q = (ln_g[:, None] * f(inputs["W_q"])).reshape(IN, H, D)
    W_kv = (ln_g[:, None] * f(inputs["W_kv"])).reshape(IN, 2, H, D)
    W_ctx = ctx_g[:, None] * f(inputs["W_ctx"])
    W_out = f(inputs["W_out"]).reshape(H, D, IN)
    q_bias = (ln_b @ W_q.reshape(IN, H * D)).reshape(H, D)
    kv_bias = (ln_b @ W_kv.reshape(IN, 2 * H * D)).reshape(2, H, D)
    ckv_bias = ctx_b @ W_ctx + f(inputs["b_ctx"])
    common = {
        "const_ident": np.eye(128, dtype=BF),
        "const_ones": np.ones((1, 128), BF),
        "const_ones_f32": np.ones((1, 64), np.float32),
        "wctx": np.ascontiguousarray(W_ctx.astype(BF)),
        "nullkv": f(inputs["null_kv"]).astype(BF),
        "ckv_bias": np.ascontiguousarray(ckv_bias, dtype=np.float32),
        "out_g": f(inputs["out_ln_g"]).astype(BF),
        "out_b": f(inputs["out_ln_b"]).astype(BF),
    }
    in_maps = []
    for c in range(NCORES):
        b, g = c // 4, c % 4
        hs = slice(HG * g, HG * (g + 1))
        qkb = np.stack([q_bias[hs].reshape(FH)[0:128], q_bias[hs].reshape(FH)[128:256],
                        kv_bias[0, hs].reshape(FH)[0:128], kv_bias[0, hs].reshape(FH)[128:256]])
        in_maps.append({
            "x_loc": x[b].astype(BF),
            "cemb_loc": c_emb[b].astype(BF),
            "wq_loc": np.ascontiguousarray(W_q[:, hs].reshape(IN, FH).astype(BF)),
            "wk_loc": np.ascontiguousarray(W_kv[:, 0, hs].reshape(IN, FH).astype(BF)),
            "wv_loc": np.ascontiguousarray(W_kv[:, 1, hs].reshape(IN, FH).astype(BF)),
            "wout_loc": np.ascontiguousarray(W_out[hs].reshape(FH, IN).astype(BF)),
            "qk_bias": np.ascontiguousarray(qkb, dtype=np.float32),
            "cv_bias": np.ascontiguousarray(kv_bias[1, hs].reshape(1, FH).astype(BF)),
            **common,
        })
    return in_maps


def unshard(results):
    out = np.empty((B, N, IN), np.float32)
    for c in range(NCORES):
        b, r = c // 4, c % 4
        y = np.asarray(results[c]["y_out"], dtype=np.float32)
        for blk in range(3):
            t0 = BLK * blk + 128 * r
            out[b, t0 : t0 + 128, :] = y[128 * blk : 128 * (blk + 1)]
        for a in range(2):
            t0 = BLK * 3 + 256 * a + 64 * r
            y0 = 384 + 64 * a
            out[b, t0 : t0 + 64, :] = y[y0 : y0 + 64]
    return out


_CACHE = {}


def kernel(**inputs) -> np.ndarray:
    from concourse.bass_utils import run_bass_kernel_spmd

    if "nc" not in _CACHE:
        _CACHE["nc"] = build_program()
    nc = _CACHE["nc"]
    in_maps = shard_inputs(inputs)
    res = run_bass_kernel_spmd(nc, in_maps, list(range(NCORES))).results
    return unshard(res)


if __name__ == "__main__":
    nc = build_program()
    print("program built OK;",
          sum(1 for _ in nc.inst_map), "instructions")
